# revision 32
# baseline (speedup 1.0000x reference)
"""Trainium2 Bass kernel for the NiN-Conv2D problem.

Network: per-pixel MLP over 7x7x3 patches, independent per filter f:
  h0 = relu(P @ W0[:,:,f] + b0)   (147 -> 32)
  h1 = relu(h0 @ W1[:,:,f] + b1)  (32 -> 16)
  out = relu(h1 @ W2[:,:,f] + b2) (16 -> 1)
for B=32, H=W=32, F=128.

Strategy: data-parallel over batch across 8 NeuronCores (4 images each).
On each core everything runs in a "feature-major" orientation: activations
live as (d*f on partitions, pixels on free dim), weights are the stationary
matmul operand, so no transposes are needed between layers.

  L0: per group of 4 filters, lhsT = W0 chunk (K=128 full-array matmul,
      K=19+bias accumulated via 4-way row-tiled concurrent matmuls)
  L1: 64x32 PE tiling -- per pair of groups, 4 dense (k=64, m=32) matmuls
      run concurrently in distinct array tiles -> (128=8f*16, pix)
  L2: per 32-filter block, 4 accumulating block-diag matmuls, 4-way
      column-tiled -> (f, pix)

All inputs ride in ONE HBM buffer split into 7 consumption-ordered DMAs
(dispatches serialize at ~0.7us each on the Sync engine, so few+big wins).
A short burst of dummy matmuls at t=0 keeps the PE busy during the input
DMA so the HAM clock gate is warm when real work lands.

Bias+ReLU+cast(PSUM->SBUF) fused into one ACT/DVE op, split across both
engines. Matmul operands are bf16 (fp32 PSUM accumulate). b0 rides the
chunk-2 matmul via a ones-row in pt2; b1/b2 ride the evacuation ops.
"""
import numpy as np
import ml_dtypes

import concourse.bass as bass
import concourse.mybir as mybir
from concourse import bacc, tile
from concourse import bass_utils
from concourse.bass import ts

KH, KW = 7, 7
B, H, W, C, F = 32, 32, 32, 3, 128
K, D0, D1 = 147, 32, 16
NCORES = 8
BPC = B // NCORES            # 4 images per core
NPIX = BPC * H * W           # 4096 pixels per core
PTILE = 512
NT = NPIX // PTILE           # 8 pixel tiles
NWARM = 5                    # HAM warmup dummy matmuls

BF16 = mybir.dt.bfloat16
F32 = mybir.dt.float32
NPBF16 = ml_dtypes.bfloat16

# ---------------------------------------------------------------------------
# combined input buffer layout (bf16 columns). 7 tiles, one DMA each,
# ordered by first use on the device. pt2 (chunk-2 patch rows + ones row,
# 20 useful rows per 32-row band) ships separately as 4 partition-band DMAs
# so no zero rows cross the wire.
#   T0 @0     [1024]: pt1 t0 @0 | w0a q0 @512
#   T1 @1024  [1024]: w0bp
#   T2 @2048  [1536]: w0a q1..q3
#   T3 @3584  [2048]: w1bd            (dispatched after T4)
#   T4 @5632  [2048]: w0a q4..q7
#   T5 @7680  [1024]: w2bd @0 | pt1 t1 @512
#   T6 @8704  [1536]: pt1 t2..t4
#   T7 @10240 [1536]: pt1 t5..t7
# ---------------------------------------------------------------------------
TILE_OFF = [0, 1024, 2048, 3584, 5632, 7680, 8704, 10240]
TILE_LEN = [1024, 1024, 1536, 2048, 2048, 1024, 1536, 1536]
TOTC = 11776

# global pt1 slot columns (host side)
PT1COL = [0, 8192] + [8704 + 512 * i for i in range(3)] + [10240 + 512 * i for i in range(3)]
# device: (tile_idx, local col) per pixel tile
PT1LOC = [(0, 0), (5, 512), (6, 0), (6, 512), (6, 1024), (7, 0), (7, 512), (7, 1024)]


# ----------------------------------------------------------------------------
# host-side packing (layout only)
# ----------------------------------------------------------------------------

def _pack_shared(w0, b0, w1, b1, w2, b2):
    """Weight/bias blocks placed into the (128, TOTC) buffer; pt slots zero."""
    w0 = np.asarray(w0, np.float32)
    w1 = np.asarray(w1, np.float32)
    w2 = np.asarray(w2, np.float32)
    b0 = np.asarray(b0, np.float32)
    b1 = np.asarray(b1, np.float32)
    b2 = np.asarray(b2, np.float32)

    big = np.zeros((128, TOTC), np.float32)

    # w0a: per group g (4 filters), (K=147 -> first 128 rows) as (128, 128)
    # col = fl*32 + d.  w0bp: rows 128..146 + b0 row, band-packed:
    # group 4q+r lives at partitions 32r..32r+19, cols 128q..128q+128.
    w0a = np.empty((128, 32, 128), np.float32)
    w0bp = np.zeros((128, 8, 128), np.float32)
    for g in range(32):
        m = w0[:, :, 4 * g:4 * g + 4].transpose(0, 2, 1).reshape(K, 128)
        w0a[:, g, :] = m[:128]
        q, r = divmod(g, 4)
        w0bp[32 * r:32 * r + 19, q, :] = m[128:]
        w0bp[32 * r + 19, q, :] = b0[:, 4 * g:4 * g + 4].T.reshape(128)

    big[:, 512:1024] = w0a[:, 0:4].reshape(128, 512)
    big[:, 2048:3584] = w0a[:, 4:16].reshape(128, 1536)
    big[:, 5632:7680] = w0a[:, 16:32].reshape(128, 2048)
    big[:, 1024:2048] = w0bp.reshape(128, 1024)

    # w1bd: block-diag (128, 64) per pair of groups -- k=128 keeps the
    # moving-stream xbus fully utilized (4-way 64-row tiling measured slower)
    w1bd = np.zeros((128, 32, 64), np.float32)
    for g in range(32):
        for fl in range(4):
            f = 4 * g + fl
            w1bd[fl * 32:(fl + 1) * 32, g, fl * 16:(fl + 1) * 16] = w1[:, :, f]
    big[:, 3584:5632] = w1bd.reshape(128, 2048)

    # w2bd: [k=half*64+fl*16+d1, pair, col]
    w2bd = np.zeros((128, 16, 32), np.float32)
    for p in range(16):
        for half in range(2):
            for fl in range(4):
                f = 8 * p + half * 4 + fl
                col = f - 32 * (p // 4)
                w2bd[half * 64 + fl * 16:half * 64 + (fl + 1) * 16, p, col] = w2[:, 0, f]
    big[:, 7680:8192] = w2bd.reshape(128, 512)

    # biases, fp32 (DVE tensor_scalar requires an fp32 scalar AP)
    bias = np.empty((128, 17), np.float32)
    for p in range(16):
        for half in range(2):
            g = 2 * p + half
            bias[half * 64:(half + 1) * 64, p] = b1[:, 4 * g:4 * g + 4].T.reshape(64)
    bias[:, 16] = b2.reshape(128)

    return big.astype(NPBF16), bias


def _pack_pt2(PT):
    """pt2 band data: rows 20r..20r+18 = patch rows 128..146, row 20r+19 = ones
    (carries b0 through the matmul)."""
    p2 = np.empty((80, NPIX), np.float32)
    for r in range(4):
        p2[20 * r:20 * r + 19] = PT[128:]
        p2[20 * r + 19] = 1.0
    return p2.astype(NPBF16)


def _im2col_T(x_core):
    """x_core (4,32,32,3) fp32 -> PT (147, 4096) with k=(kh*7+kw)*3+c."""
    xp = np.pad(np.asarray(x_core, np.float32), ((0, 0), (3, 3), (3, 3), (0, 0)))
    PT = np.empty((K, NPIX), np.float32)
    for kh in range(KH):
        for kw in range(KW):
            blk = xp[:, kh:kh + H, kw:kw + W, :]
            t = kh * 7 + kw
            PT[t * 3:t * 3 + 3] = blk.transpose(3, 0, 1, 2).reshape(3, NPIX)
    return PT


# ----------------------------------------------------------------------------
# device kernel
# ----------------------------------------------------------------------------

def _body(tc):
    nc = tc.nc
    Relu = mybir.ActivationFunctionType.Relu
    Add, Max = mybir.AluOpType.add, mybir.AluOpType.max

    big_d = nc.dram_tensor("big", [128, TOTC], BF16, kind="ExternalInput").ap()
    pt2_d = nc.dram_tensor("pt2", [80, NPIX], BF16, kind="ExternalInput").ap()
    bias_d = nc.dram_tensor("bias", [128, 17], F32, kind="ExternalInput").ap()
    out = nc.dram_tensor("out", [128, NPIX], BF16, kind="ExternalOutput").ap()

    with (
        tc.tile_pool(name="consts", bufs=1) as cpool,
        tc.tile_pool(name="h0", bufs=34) as h0pool,
        tc.tile_pool(name="h1", bufs=20) as h1pool,
        tc.tile_pool(name="outs", bufs=3) as opool,
        tc.tile_pool(name="l0p", bufs=3, space="PSUM") as l0pool,
        tc.tile_pool(name="l12p", bufs=2, space="PSUM") as l12pool,
    ):
        # ---- HAM warmup: PE busy from t=0 so the clock gate is at 8/8 when
        # the first real matmul's inputs land.
        wu = cpool.tile([128, 512], BF16, tag="wu")
        nc.gpsimd.memset(wu[:], 0.0)
        wups = l12pool.tile([128, PTILE], F32, tag="l12")
        for _ in range(NWARM):
            nc.tensor.matmul(wups[:], wu[:, 0:128], wu[:], start=True, stop=True)

        # ---- input DMAs: one per layout tile, in consumption order
        # (w0a q4..7 before w1bd: tile 0's L0 unblocks sooner).
        T = [None] * 8
        pt2t = cpool.tile([128, NPIX], BF16, tag="pt2")
        for i in range(8):
            T[i] = cpool.tile([128, TILE_LEN[i]], BF16, tag=f"in{i}", name=f"in{i}")
        bias_t = cpool.tile([128, 17], F32, tag="bias")

        def load(i):
            nc.sync.dma_start(T[i][:], big_d[:, TILE_OFF[i]:TILE_OFF[i] + TILE_LEN[i]])
        load(0)
        load(1)
        for r in range(4):
            nc.sync.dma_start(pt2t[32 * r:32 * r + 20, :],
                              pt2_d[20 * r:20 * r + 20, :])
        nc.sync.dma_start(bias_t[:], bias_d[:])
        load(2)
        load(4)
        load(3)
        load(5)
        load(6)
        load(7)

        def w0a_ap(g):
            q, r = divmod(g, 4)
            if q == 0:
                return T[0][:, 512 + 128 * r:640 + 128 * r]
            if q <= 3:
                c = 512 * (q - 1) + 128 * r
                return T[2][:, c:c + 128]
            c = 512 * (q - 4) + 128 * r
            return T[4][:, c:c + 128]

        def pt1_ap(t):
            i, c = PT1LOC[t]
            return T[i][:, c:c + PTILE]

        def pt2_ap(t, r):
            return pt2t[32 * r:32 * r + 20, ts(t, PTILE)]

        def w0bp_ap(q, r):
            return T[1][32 * r:32 * r + 20, 128 * q:128 * q + 128]

        def w1bd_ap(g):
            return T[3][:, 64 * g:64 * g + 64]

        def w2_ap(p):
            return T[5][:, 32 * p:32 * p + 32]

        def b1_ap(p):
            return bias_t[:, p:p + 1]

        b2_ap = lambda: bias_t[:, 16:17]

        def emit_l0(t):
            # ---- layer 0: 8 quads of 4 filter-groups; two (128,1024) PSUM
            # tiles per quad. chunk1 = K rows 0..127 full-array; chunk2
            # (K rows 128..146 + bias row) = 4-way row-tiled concurrent
            # matmuls (4 bands x 4 banks). Bias rides in the matmul.
            h0 = []       # 16 tiles (128,1024): groups (2j, 2j+1)
            for q in range(8):
                psA = l0pool.tile([128, 2 * PTILE], F32, tag="l0")
                psB = l0pool.tile([128, 2 * PTILE], F32, tag="l0")
                for r in range(4):
                    ps = psA if r < 2 else psB
                    dst = ps[:, ts(r % 2, PTILE)]
                    nc.tensor.matmul(dst, w0a_ap(4 * q + r), pt1_ap(t),
                                     start=True, stop=False)
                for r in range(4):
                    ps = psA if r < 2 else psB
                    dst = ps[:, ts(r % 2, PTILE)]
                    nc.tensor.matmul(dst, w0bp_ap(q, r), pt2_ap(t, r),
                                     start=False, stop=True,
                                     tile_position=(32 * r, 0))
                if t == 0 and q < 6:
                    # keep the PE (and the HAM clock gate) busy while tile-0
                    # quads wait on their input DMAs
                    for _ in range(2):
                        nc.tensor.matmul(wups[:], wu[:, 0:128], wu[:],
                                         start=True, stop=True)
                for j, ps in ((2 * q, psA), (2 * q + 1, psB)):
                    h = h0pool.tile([128, 2 * PTILE], BF16, tag="h0")
                    if j % 2 == 0:
                        nc.scalar.activation(h[:], ps[:], Relu)
                    else:
                        nc.vector.tensor_scalar_max(h[:], ps[:], 0.0)
                    h0.append(h)
            return h0

        def emit_l12(t, h0):
            pix = ts(t, PTILE)
            # ---- layer 1: per pair of groups, block-diag W1 (128, 64), two
            # col-tiled matmuls fill the two partition halves of one PSUM bank
            h1 = []
            for p in range(16):
                ps = l12pool.tile([128, PTILE], F32, tag="l12")
                nc.tensor.matmul(ps[0:64, :], w1bd_ap(2 * p),
                                 h0[p][:, 0:PTILE], start=True, stop=True)
                nc.tensor.matmul(ps[64:128, :], w1bd_ap(2 * p + 1),
                                 h0[p][:, PTILE:], start=True, stop=True)
                hh = h1pool.tile([128, PTILE], BF16, tag="h1")
                if p % 2 == 0:
                    nc.scalar.activation(hh[:], ps[:], Relu, bias=b1_ap(p))
                else:
                    nc.vector.tensor_scalar(hh[:], ps[:], b1_ap(p), 0.0, Add, Max)
                h1.append(hh)
            # ---- layer 2: 4 blocks of 32 filters; q-major order so the 4
            # blocks' matmuls hit disjoint PE column groups concurrently
            ps2 = l12pool.tile([128, PTILE], F32, tag="l12")
            for qq in range(4):
                for jj in range(4):
                    p = 4 * jj + qq
                    nc.tensor.matmul(ps2[32 * jj:32 * jj + 32, :],
                                     w2_ap(p), h1[p][:],
                                     start=(qq == 0), stop=(qq == 3),
                                     tile_position=(0, 32 * jj))
            ot = opool.tile([128, PTILE], BF16, tag="o")
            if t == NT - 1:
                # split the final evac + store so the kernel-ending DMA is
                # small (the postamble waits on its completion semaphore)
                HALF = PTILE // 2
                nc.scalar.activation(ot[:, 0:HALF], ps2[:, 0:HALF], Relu, bias=b2_ap())
                nc.sync.dma_start(out[:, t * PTILE:t * PTILE + HALF], ot[:, 0:HALF])
                nc.scalar.activation(ot[:, HALF:], ps2[:, HALF:], Relu, bias=b2_ap())
                nc.sync.dma_start(out[:, t * PTILE + HALF:(t + 1) * PTILE], ot[:, HALF:])
            else:
                nc.scalar.activation(ot[:], ps2[:], Relu, bias=b2_ap())
                nc.sync.dma_start(out[:, pix], ot[:])

        # ---- software-pipelined emission: L0 of tile t, then L1+L2 of tile
        # t-1. The evacuation engines then see tile t's h0 evacs before tile
        # t-1's h1 work, so the L0 PSUM rotation never waits, and L1 matmuls
        # read h0 tiles whose evacuation finished a full tile ago.
        prev_h0 = None
        for t in range(NT):
            cur_h0 = emit_l0(t)
            if prev_h0 is not None:
                emit_l12(t - 1, prev_h0)
            prev_h0 = cur_h0
        emit_l12(NT - 1, prev_h0)


_COMPILED = None


def _get_compiled():
    global _COMPILED
    if _COMPILED is None:
        import time as _time
        t0 = _time.time()
        nc = bacc.Bacc("TRN2", target_bir_lowering=False, debug=False,
                       num_devices=NCORES)
        with tile.TileContext(nc) as tc:
            _body(tc)
        t1 = _time.time()
        nc.compile()
        t2 = _time.time()
        print(f"[kernel] tile build+schedule {t1 - t0:.1f}s, bacc compile {t2 - t1:.1f}s",
              flush=True)
        _COMPILED = nc
    return _COMPILED


# ----------------------------------------------------------------------------
# public entry point
# ----------------------------------------------------------------------------

def kernel(x, w0, b0, w1, b1, w2, b2, _trace=False):
    x = np.asarray(x, np.float32)
    shared, bias = _pack_shared(w0, b0, w1, b1, w2, b2)

    in_maps = []
    for k in range(NCORES):
        PT = _im2col_T(x[BPC * k:BPC * (k + 1)])
        big = shared.copy()
        pt1 = PT[:128].astype(NPBF16)
        for t in range(NT):
            big[:, PT1COL[t]:PT1COL[t] + PTILE] = pt1[:, ts_np(t)]
        in_maps.append({"big": big, "pt2": _pack_pt2(PT), "bias": bias})

    import time as _time
    nc = _get_compiled()
    t0 = _time.time()
    res = bass_utils.run_bass_kernel_spmd(
        nc, in_maps, core_ids=list(range(NCORES)), trace=_trace)
    print(f"[kernel] run_bass_kernel_spmd {_time.time() - t0:.1f}s", flush=True)

    outs = []
    for k in range(NCORES):
        oc = np.asarray(res.results[k]["out"], np.float32)   # (128, 4096)
        outs.append(oc.reshape(F, BPC, H, W).transpose(1, 2, 3, 0))
    full = np.concatenate(outs, axis=0).astype(np.float32)
    if _trace:
        return full, res
    return full


def ts_np(t):
    return slice(t * PTILE, (t + 1) * PTILE)


# revision 33
# speedup vs baseline: 1.0349x; 1.0349x over previous
"""Trainium2 Bass kernel for the NiN-Conv2D problem.

Network: per-pixel MLP over 7x7x3 patches, independent per filter f:
  h0 = relu(P @ W0[:,:,f] + b0)   (147 -> 32)
  h1 = relu(h0 @ W1[:,:,f] + b1)  (32 -> 16)
  out = relu(h1 @ W2[:,:,f] + b2) (16 -> 1)
for B=32, H=W=32, F=128.

Strategy: data-parallel over batch across 8 NeuronCores (4 images each).
On each core everything runs in a "feature-major" orientation: activations
live as (d*f on partitions, pixels on free dim), weights are the stationary
matmul operand, so no transposes are needed between layers.

  L0: per group of 4 filters, lhsT = W0 chunk (K=128 full-array matmul,
      K=19+bias accumulated via 4-way row-tiled concurrent matmuls)
  L1: 64x32 PE tiling -- per pair of groups, 4 dense (k=64, m=32) matmuls
      run concurrently in distinct array tiles -> (128=8f*16, pix)
  L2: per 32-filter block, 4 accumulating block-diag matmuls, 4-way
      column-tiled -> (f, pix)

All inputs ride in ONE HBM buffer split into 7 consumption-ordered DMAs
(dispatches serialize at ~0.7us each on the Sync engine, so few+big wins).
A short burst of dummy matmuls at t=0 keeps the PE busy during the input
DMA so the HAM clock gate is warm when real work lands.

Bias+ReLU+cast(PSUM->SBUF) fused into one ACT/DVE op, split across both
engines. Matmul operands are bf16 (fp32 PSUM accumulate). b0 rides the
chunk-2 matmul via a ones-row in pt2; b1/b2 ride the evacuation ops.
"""
import numpy as np
import ml_dtypes

import concourse.bass as bass
import concourse.mybir as mybir
from concourse import bacc, tile
from concourse import bass_utils
from concourse.bass import ts

KH, KW = 7, 7
B, H, W, C, F = 32, 32, 32, 3, 128
K, D0, D1 = 147, 32, 16
NCORES = 8
BPC = B // NCORES            # 4 images per core
NPIX = BPC * H * W           # 4096 pixels per core
PTILE = 512
NT = NPIX // PTILE           # 8 pixel tiles
NWARM = 5                    # HAM warmup dummy matmuls

BF16 = mybir.dt.bfloat16
F32 = mybir.dt.float32
NPBF16 = ml_dtypes.bfloat16

# ---------------------------------------------------------------------------
# combined input buffer layout (bf16 columns). 7 tiles, one DMA each,
# ordered by first use on the device. pt2 (chunk-2 patch rows + ones row,
# 20 useful rows per 32-row band) ships separately as 4 partition-band DMAs
# so no zero rows cross the wire.
#   T0 @0     [1024]: pt1 t0 @0 | w0a q0 @512
#   T1 @1024  [1024]: w0bp
#   T2 @2048  [1536]: w0a q1..q3
#   T3 @3584  [2048]: w1bd            (dispatched after T4)
#   T4 @5632  [2048]: w0a q4..q7
#   T5 @7680  [1024]: w2bd @0 | pt1 t1 @512
#   T6 @8704  [1536]: pt1 t2..t4
#   T7 @10240 [1536]: pt1 t5..t7
# ---------------------------------------------------------------------------
TILE_OFF = [0, 1024, 2048, 3584, 5632, 7680, 8704, 10240]
TILE_LEN = [1024, 1024, 1536, 2048, 2048, 1024, 1536, 1536]
TOTC = 11776

# global pt1 slot columns (host side)
PT1COL = [0, 8192] + [8704 + 512 * i for i in range(3)] + [10240 + 512 * i for i in range(3)]
# device: (tile_idx, local col) per pixel tile
PT1LOC = [(0, 0), (5, 512), (6, 0), (6, 512), (6, 1024), (7, 0), (7, 512), (7, 1024)]


# ----------------------------------------------------------------------------
# host-side packing (layout only)
# ----------------------------------------------------------------------------

def _pack_shared(w0, b0, w1, b1, w2, b2):
    """Weight/bias blocks placed into the (128, TOTC) buffer; pt slots zero."""
    w0 = np.asarray(w0, np.float32)
    w1 = np.asarray(w1, np.float32)
    w2 = np.asarray(w2, np.float32)
    b0 = np.asarray(b0, np.float32)
    b1 = np.asarray(b1, np.float32)
    b2 = np.asarray(b2, np.float32)

    big = np.zeros((128, TOTC), np.float32)

    # w0a: per group g (4 filters), (K=147 -> first 128 rows) as (128, 128)
    # col = fl*32 + d.  w0bp: rows 128..146 + b0 row, band-packed:
    # group 4q+r lives at partitions 32r..32r+19, cols 128q..128q+128.
    w0a = np.empty((128, 32, 128), np.float32)
    w0bp = np.zeros((128, 8, 128), np.float32)
    for g in range(32):
        m = w0[:, :, 4 * g:4 * g + 4].transpose(0, 2, 1).reshape(K, 128)
        w0a[:, g, :] = m[:128]
        q, r = divmod(g, 4)
        w0bp[32 * r:32 * r + 19, q, :] = m[128:]
        w0bp[32 * r + 19, q, :] = b0[:, 4 * g:4 * g + 4].T.reshape(128)

    big[:, 512:1024] = w0a[:, 0:4].reshape(128, 512)
    big[:, 2048:3584] = w0a[:, 4:16].reshape(128, 1536)
    big[:, 5632:7680] = w0a[:, 16:32].reshape(128, 2048)
    big[:, 1024:2048] = w0bp.reshape(128, 1024)

    # w1bd: block-diag (128, 64) per pair of groups -- k=128 keeps the
    # moving-stream xbus fully utilized (4-way 64-row tiling measured slower)
    w1bd = np.zeros((128, 32, 64), np.float32)
    for g in range(32):
        for fl in range(4):
            f = 4 * g + fl
            w1bd[fl * 32:(fl + 1) * 32, g, fl * 16:(fl + 1) * 16] = w1[:, :, f]
    big[:, 3584:5632] = w1bd.reshape(128, 2048)

    # w2bd: [k=half*64+fl*16+d1, pair, col]
    w2bd = np.zeros((128, 16, 32), np.float32)
    for p in range(16):
        for half in range(2):
            for fl in range(4):
                f = 8 * p + half * 4 + fl
                col = f - 32 * (p // 4)
                w2bd[half * 64 + fl * 16:half * 64 + (fl + 1) * 16, p, col] = w2[:, 0, f]
    big[:, 7680:8192] = w2bd.reshape(128, 512)

    # biases, fp32 (DVE tensor_scalar requires an fp32 scalar AP)
    bias = np.empty((128, 17), np.float32)
    for p in range(16):
        for half in range(2):
            g = 2 * p + half
            bias[half * 64:(half + 1) * 64, p] = b1[:, 4 * g:4 * g + 4].T.reshape(64)
    bias[:, 16] = b2.reshape(128)

    return big.astype(NPBF16), bias


def _pack_pt2(PT):
    """pt2 band data: rows 20r..20r+18 = patch rows 128..146, row 20r+19 = ones
    (carries b0 through the matmul)."""
    p2 = np.empty((80, NPIX), np.float32)
    for r in range(4):
        p2[20 * r:20 * r + 19] = PT[128:]
        p2[20 * r + 19] = 1.0
    return p2.astype(NPBF16)


def _im2col_T(x_core):
    """x_core (4,32,32,3) fp32 -> PT (147, 4096) with k=(kh*7+kw)*3+c."""
    xp = np.pad(np.asarray(x_core, np.float32), ((0, 0), (3, 3), (3, 3), (0, 0)))
    PT = np.empty((K, NPIX), np.float32)
    for kh in range(KH):
        for kw in range(KW):
            blk = xp[:, kh:kh + H, kw:kw + W, :]
            t = kh * 7 + kw
            PT[t * 3:t * 3 + 3] = blk.transpose(3, 0, 1, 2).reshape(3, NPIX)
    return PT


# ----------------------------------------------------------------------------
# device kernel
# ----------------------------------------------------------------------------

def _body(tc):
    nc = tc.nc
    Relu = mybir.ActivationFunctionType.Relu
    Add, Max = mybir.AluOpType.add, mybir.AluOpType.max

    big_d = nc.dram_tensor("big", [128, TOTC], BF16, kind="ExternalInput").ap()
    pt2_d = nc.dram_tensor("pt2", [80, NPIX], BF16, kind="ExternalInput").ap()
    bias_d = nc.dram_tensor("bias", [128, 17], F32, kind="ExternalInput").ap()
    out = nc.dram_tensor("out", [128, NPIX], BF16, kind="ExternalOutput").ap()

    with (
        tc.tile_pool(name="consts", bufs=1) as cpool,
        tc.tile_pool(name="h0", bufs=34) as h0pool,
        tc.tile_pool(name="h1", bufs=20) as h1pool,
        tc.tile_pool(name="outs", bufs=3) as opool,
        tc.tile_pool(name="l0p", bufs=3, space="PSUM") as l0pool,
        tc.tile_pool(name="l12p", bufs=2, space="PSUM") as l12pool,
    ):
        # ---- HAM warmup: PE busy from t=0 so the clock gate is at 8/8 when
        # the first real matmul's inputs land.
        wu = cpool.tile([128, 512], BF16, tag="wu")
        nc.gpsimd.memset(wu[:], 0.0)
        wups = l12pool.tile([128, PTILE], F32, tag="l12")
        for _ in range(NWARM):
            nc.tensor.matmul(wups[:], wu[:, 0:128], wu[:], start=True, stop=True)

        # ---- input DMAs: one per layout tile, in consumption order
        # (w0a q4..7 before w1bd: tile 0's L0 unblocks sooner).
        T = [None] * 8
        pt2t = cpool.tile([128, NPIX], BF16, tag="pt2")
        for i in range(8):
            T[i] = cpool.tile([128, TILE_LEN[i]], BF16, tag=f"in{i}", name=f"in{i}")
        bias_t = cpool.tile([128, 17], F32, tag="bias")

        def load(i):
            nc.sync.dma_start(T[i][:], big_d[:, TILE_OFF[i]:TILE_OFF[i] + TILE_LEN[i]])
        load(0)
        load(1)
        for r in range(4):
            nc.sync.dma_start(pt2t[32 * r:32 * r + 20, :],
                              pt2_d[20 * r:20 * r + 20, :])
        nc.sync.dma_start(bias_t[:], bias_d[:])
        load(2)
        load(4)
        load(3)
        load(5)
        load(6)
        load(7)

        def w0a_ap(g):
            q, r = divmod(g, 4)
            if q == 0:
                return T[0][:, 512 + 128 * r:640 + 128 * r]
            if q <= 3:
                c = 512 * (q - 1) + 128 * r
                return T[2][:, c:c + 128]
            c = 512 * (q - 4) + 128 * r
            return T[4][:, c:c + 128]

        def pt1_ap(t):
            i, c = PT1LOC[t]
            return T[i][:, c:c + PTILE]

        def pt2_ap(t, r):
            return pt2t[32 * r:32 * r + 20, ts(t, PTILE)]

        def w0bp_ap(q, r):
            return T[1][32 * r:32 * r + 20, 128 * q:128 * q + 128]

        def w1bd_ap(g):
            return T[3][:, 64 * g:64 * g + 64]

        def w2_ap(p):
            return T[5][:, 32 * p:32 * p + 32]

        def b1_ap(p):
            return bias_t[:, p:p + 1]

        b2_ap = lambda: bias_t[:, 16:17]

        def emit_l0(t):
            # ---- layer 0: 8 quads of 4 filter-groups; two (128,1024) PSUM
            # tiles per quad. chunk1 = K rows 0..127 full-array; chunk2
            # (K rows 128..146 + bias row) = 4-way row-tiled concurrent
            # matmuls (4 bands x 4 banks). Bias rides in the matmul.
            h0 = []       # 16 tiles (128,1024): groups (2j, 2j+1)
            for q in range(8):
                psA = l0pool.tile([128, 2 * PTILE], F32, tag="l0")
                psB = l0pool.tile([128, 2 * PTILE], F32, tag="l0")
                for r in range(4):
                    ps = psA if r < 2 else psB
                    dst = ps[:, ts(r % 2, PTILE)]
                    nc.tensor.matmul(dst, w0a_ap(4 * q + r), pt1_ap(t),
                                     start=True, stop=False)
                for r in range(4):
                    ps = psA if r < 2 else psB
                    dst = ps[:, ts(r % 2, PTILE)]
                    nc.tensor.matmul(dst, w0bp_ap(q, r), pt2_ap(t, r),
                                     start=False, stop=True,
                                     tile_position=(32 * r, 0))
                if t == 0 and q < 6:
                    # keep the PE (and the HAM clock gate) busy while tile-0
                    # quads wait on their input DMAs
                    for _ in range(2):
                        nc.tensor.matmul(wups[:], wu[:, 0:128], wu[:],
                                         start=True, stop=True)
                for j, ps in ((2 * q, psA), (2 * q + 1, psB)):
                    h = h0pool.tile([128, 2 * PTILE], BF16, tag="h0")
                    if j % 2 == 0:
                        nc.scalar.activation(h[:], ps[:], Relu)
                    else:
                        nc.vector.tensor_scalar_max(h[:], ps[:], 0.0)
                    h0.append(h)
            return h0

        def emit_l12(t, h0):
            pix = ts(t, PTILE)
            # ---- layer 1: per pair of groups, block-diag W1 (128, 64), two
            # col-tiled matmuls fill the two partition halves of one PSUM bank
            h1 = []
            for p in range(16):
                ps = l12pool.tile([128, PTILE], F32, tag="l12")
                nc.tensor.matmul(ps[0:64, :], w1bd_ap(2 * p),
                                 h0[p][:, 0:PTILE], start=True, stop=True)
                nc.tensor.matmul(ps[64:128, :], w1bd_ap(2 * p + 1),
                                 h0[p][:, PTILE:], start=True, stop=True)
                hh = h1pool.tile([128, PTILE], BF16, tag="h1")
                if p % 2 == 0:
                    nc.scalar.activation(hh[:], ps[:], Relu, bias=b1_ap(p))
                else:
                    nc.vector.tensor_scalar(hh[:], ps[:], b1_ap(p), 0.0, Add, Max)
                h1.append(hh)
            # ---- layer 2: 4 blocks of 32 filters; q-major order so the 4
            # blocks' matmuls hit disjoint PE column groups concurrently
            ps2 = l12pool.tile([128, PTILE], F32, tag="l12")
            for qq in range(4):
                for jj in range(4):
                    p = 4 * jj + qq
                    nc.tensor.matmul(ps2[32 * jj:32 * jj + 32, :],
                                     w2_ap(p), h1[p][:],
                                     start=(qq == 0), stop=(qq == 3),
                                     tile_position=(0, 32 * jj))
            ot = opool.tile([128, PTILE], BF16, tag="o")
            if t == NT - 1:
                # split the final evac + store so the kernel-ending DMA is
                # small (the postamble waits on its completion semaphore)
                HALF = PTILE // 2
                nc.scalar.activation(ot[:, 0:HALF], ps2[:, 0:HALF], Relu, bias=b2_ap())
                nc.sync.dma_start(out[:, t * PTILE:t * PTILE + HALF], ot[:, 0:HALF])
                nc.scalar.activation(ot[:, HALF:], ps2[:, HALF:], Relu, bias=b2_ap())
                nc.sync.dma_start(out[:, t * PTILE + HALF:(t + 1) * PTILE], ot[:, HALF:])
            else:
                nc.scalar.activation(ot[:], ps2[:], Relu, bias=b2_ap())
                nc.sync.dma_start(out[:, pix], ot[:])

        for t in range(NT):
            emit_l12(t, emit_l0(t))


_COMPILED = None


def _get_compiled():
    global _COMPILED
    if _COMPILED is None:
        import time as _time
        t0 = _time.time()
        nc = bacc.Bacc("TRN2", target_bir_lowering=False, debug=False,
                       num_devices=NCORES)
        with tile.TileContext(nc) as tc:
            _body(tc)
        t1 = _time.time()
        nc.compile()
        t2 = _time.time()
        print(f"[kernel] tile build+schedule {t1 - t0:.1f}s, bacc compile {t2 - t1:.1f}s",
              flush=True)
        _COMPILED = nc
    return _COMPILED


# ----------------------------------------------------------------------------
# public entry point
# ----------------------------------------------------------------------------

def kernel(x, w0, b0, w1, b1, w2, b2, _trace=False):
    x = np.asarray(x, np.float32)
    shared, bias = _pack_shared(w0, b0, w1, b1, w2, b2)

    in_maps = []
    for k in range(NCORES):
        PT = _im2col_T(x[BPC * k:BPC * (k + 1)])
        big = shared.copy()
        pt1 = PT[:128].astype(NPBF16)
        for t in range(NT):
            big[:, PT1COL[t]:PT1COL[t] + PTILE] = pt1[:, ts_np(t)]
        in_maps.append({"big": big, "pt2": _pack_pt2(PT), "bias": bias})

    import time as _time
    nc = _get_compiled()
    t0 = _time.time()
    res = bass_utils.run_bass_kernel_spmd(
        nc, in_maps, core_ids=list(range(NCORES)), trace=_trace)
    print(f"[kernel] run_bass_kernel_spmd {_time.time() - t0:.1f}s", flush=True)

    outs = []
    for k in range(NCORES):
        oc = np.asarray(res.results[k]["out"], np.float32)   # (128, 4096)
        outs.append(oc.reshape(F, BPC, H, W).transpose(1, 2, 3, 0))
    full = np.concatenate(outs, axis=0).astype(np.float32)
    if _trace:
        return full, res
    return full


def ts_np(t):
    return slice(t * PTILE, (t + 1) * PTILE)


# revision 73
# speedup vs baseline: 1.1013x; 1.0642x over previous
"""Trainium2 Bass kernel for the NiN-Conv2D problem.

Network: per-pixel MLP over 7x7x3 patches, independent per filter f:
  h0 = relu(P @ W0[:,:,f] + b0)   (147 -> 32)
  h1 = relu(h0 @ W1[:,:,f] + b1)  (32 -> 16)
  out = relu(h1 @ W2[:,:,f] + b2) (16 -> 1)
for B=32, H=W=32, F=128.

Strategy: data-parallel over batch across 8 NeuronCores (4 images each).
On each core everything runs in a "feature-major" orientation: activations
live as (d*f on partitions, pixels on free dim), weights are the stationary
matmul operand, so no transposes are needed between layers.

  L0: per group of 4 filters, lhsT = W0 chunk (K=128 full-array matmul,
      K=19+bias accumulated via 4-way row-tiled concurrent matmuls)
  L1: per pair of groups, block-diag W1 (128, 64); two col-tiled matmuls
      (concurrent) fill the two partition halves of one PSUM bank
  L2: per 32-filter block, 4 accumulating block-diag matmuls, 4-way
      column-tiled -> (f, pix)

All inputs ride in ONE HBM buffer split into 7 consumption-ordered DMAs
(dispatches serialize at ~0.7us each on the Sync engine, so few+big wins).
A short burst of dummy matmuls at t=0 keeps the PE busy during the input
DMA so the HAM clock gate is warm when real work lands.

Bias+ReLU+cast(PSUM->SBUF) fused into one ACT/DVE op, split across both
engines. Matmul operands are bf16 (fp32 PSUM accumulate). b0 rides the
chunk-2 matmul via a ones-row in pt2; b1/b2 ride the evacuation ops.
"""
import numpy as np
import ml_dtypes

import concourse.bass as bass
import concourse.mybir as mybir
from concourse import bacc, tile
from concourse import bass_utils
from concourse.bass import ts

KH, KW = 7, 7
B, H, W, C, F = 32, 32, 32, 3, 128
K, D0, D1 = 147, 32, 16
NCORES = 8
BPC = B // NCORES            # 4 images per core
NPIX = BPC * H * W           # 4096 pixels per core
PTILE = 512
NT = NPIX // PTILE           # 8 pixel tiles
NWARM = 5                    # HAM warmup dummy matmuls

BF16 = mybir.dt.bfloat16
F32 = mybir.dt.float32
NPBF16 = ml_dtypes.bfloat16

# ---------------------------------------------------------------------------
# combined input buffer layout (bf16 columns). One tile per DMA, ordered by
# first use on the device. pt2 (chunk-2 patch rows + ones row, 20 useful
# rows per 32-row band) ships separately as 4 partition-band DMAs so no
# zero rows cross the wire.
#   T0 @0     [1024]: pt1 t0 @0 | w0a q0 @512
#   T1 @1024  [1024]: w0bp
#   T2 @2048  [1536]: w0a q1..q3
#   T3 @3584  [2048]: w1bd            (dispatched after T4)
#   T4 @5632  [2048]: w0a q4..q7
#   T5 @7680  [1024]: w2bd @0 | pt1 t1 @512
#   T6 @8704  [1536]: pt1 t2..t4
#   T7 @10240 [1536]: pt1 t5..t7
# ---------------------------------------------------------------------------
TILE_OFF = [0, 1024, 2048, 3584, 5632, 7680, 8704, 10240]
TILE_LEN = [1024, 1024, 1536, 2048, 2048, 1024, 1536, 1536]
TOTC = 11776

# global pt1 slot columns (host side)
PT1COL = [0, 8192] + [8704 + 512 * i for i in range(3)] + [10240 + 512 * i for i in range(3)]
# device: (tile_idx, local col) per pixel tile
PT1LOC = [(0, 0), (5, 512), (6, 0), (6, 512), (6, 1024), (7, 0), (7, 512), (7, 1024)]


# ----------------------------------------------------------------------------
# host-side packing (layout only)
# ----------------------------------------------------------------------------

def _pack_shared(w0, b0, w1, b1, w2, b2):
    """Weight/bias blocks placed into the (128, TOTC) buffer; pt slots zero."""
    w0 = np.asarray(w0, np.float32)
    w1 = np.asarray(w1, np.float32)
    w2 = np.asarray(w2, np.float32)
    b0 = np.asarray(b0, np.float32)
    b1 = np.asarray(b1, np.float32)
    b2 = np.asarray(b2, np.float32)

    big = np.zeros((128, TOTC), np.float32)

    # w0a: per group g (4 filters), (K=147 -> first 128 rows) as (128, 128)
    # col = fl*32 + d.  w0bp: rows 128..146 + b0 row, band-packed:
    # group 4q+r lives at partitions 32r..32r+19, cols 128q..128q+128.
    w0a = np.empty((128, 32, 128), np.float32)
    w0bp = np.zeros((128, 8, 128), np.float32)
    for g in range(32):
        m = w0[:, :, 4 * g:4 * g + 4].transpose(0, 2, 1).reshape(K, 128)
        w0a[:, g, :] = m[:128]
        q, r = divmod(g, 4)
        w0bp[32 * r:32 * r + 19, q, :] = m[128:]
        w0bp[32 * r + 19, q, :] = b0[:, 4 * g:4 * g + 4].T.reshape(128)

    big[:, 512:1024] = w0a[:, 0:4].reshape(128, 512)
    big[:, 2048:3584] = w0a[:, 4:16].reshape(128, 1536)
    big[:, 5632:7680] = w0a[:, 16:32].reshape(128, 2048)
    big[:, 1024:2048] = w0bp.reshape(128, 1024)

    # w1bd: block-diag (128, 64) per pair of groups -- k=128 keeps the
    # moving-stream xbus fully utilized (4-way 64-row tiling measured slower)
    w1bd = np.zeros((128, 32, 64), np.float32)
    for g in range(32):
        for fl in range(4):
            f = 4 * g + fl
            w1bd[fl * 32:(fl + 1) * 32, g, fl * 16:(fl + 1) * 16] = w1[:, :, f]
    big[:, 3584:5632] = w1bd.reshape(128, 2048)

    # w2bd: [k=half*64+fl*16+d1, pair, col]
    w2bd = np.zeros((128, 16, 32), np.float32)
    for p in range(16):
        for half in range(2):
            for fl in range(4):
                f = 8 * p + half * 4 + fl
                col = f - 32 * (p // 4)
                w2bd[half * 64 + fl * 16:half * 64 + (fl + 1) * 16, p, col] = w2[:, 0, f]
    big[:, 7680:8192] = w2bd.reshape(128, 512)

    # biases, fp32 (DVE tensor_scalar requires an fp32 scalar AP)
    bias = np.empty((128, 17), np.float32)
    for p in range(16):
        for half in range(2):
            g = 2 * p + half
            bias[half * 64:(half + 1) * 64, p] = b1[:, 4 * g:4 * g + 4].T.reshape(64)
    bias[:, 16] = b2.reshape(128)

    return big.astype(NPBF16), bias


def _pack_pt2(PT):
    """pt2 band data: rows 20r..20r+18 = patch rows 128..146, row 20r+19 = ones
    (carries b0 through the matmul)."""
    p2 = np.empty((80, NPIX), np.float32)
    for r in range(4):
        p2[20 * r:20 * r + 19] = PT[128:]
        p2[20 * r + 19] = 1.0
    return p2.astype(NPBF16)


def _im2col_T(x_core):
    """x_core (4,32,32,3) fp32 -> PT (147, 4096) with k=(kh*7+kw)*3+c."""
    xp = np.pad(np.asarray(x_core, np.float32), ((0, 0), (3, 3), (3, 3), (0, 0)))
    PT = np.empty((K, NPIX), np.float32)
    for kh in range(KH):
        for kw in range(KW):
            blk = xp[:, kh:kh + H, kw:kw + W, :]
            t = kh * 7 + kw
            PT[t * 3:t * 3 + 3] = blk.transpose(3, 0, 1, 2).reshape(3, NPIX)
    return PT


# ----------------------------------------------------------------------------
# device kernel
# ----------------------------------------------------------------------------

def _body(tc):
    nc = tc.nc
    Relu = mybir.ActivationFunctionType.Relu
    Add, Max = mybir.AluOpType.add, mybir.AluOpType.max

    big_d = nc.dram_tensor("big", [128, TOTC], BF16, kind="ExternalInput").ap()
    pt2_d = nc.dram_tensor("pt2", [80, NPIX], BF16, kind="ExternalInput").ap()
    bias_d = nc.dram_tensor("bias", [128, 17], F32, kind="ExternalInput").ap()
    out = nc.dram_tensor("out", [128, NPIX], BF16, kind="ExternalOutput").ap()

    with (
        tc.tile_pool(name="consts", bufs=1) as cpool,
        tc.tile_pool(name="h0", bufs=34) as h0pool,
        tc.tile_pool(name="h1", bufs=20) as h1pool,
        tc.tile_pool(name="outs", bufs=3) as opool,
        tc.tile_pool(name="l0p", bufs=3, space="PSUM") as l0pool,
        tc.tile_pool(name="l12p", bufs=2, space="PSUM") as l12pool,
    ):
        # ---- HAM warmup: PE busy from t=0 so the clock gate is at 8/8 when
        # the first real matmul's inputs land.
        wu = cpool.tile([128, 512], BF16, tag="wu")
        nc.gpsimd.memset(wu[:], 0.0)
        wups = l12pool.tile([128, PTILE], F32, tag="l12")
        for _ in range(NWARM):
            nc.tensor.matmul(wups[:], wu[:, 0:128], wu[:], start=True, stop=True)

        # ---- input DMAs: one per layout tile, in consumption order
        # (w0a q4..7 before w1bd: tile 0's L0 unblocks sooner).
        T = [None] * 8
        pt2t = cpool.tile([128, NPIX], BF16, tag="pt2")
        for i in range(8):
            T[i] = cpool.tile([128, TILE_LEN[i]], BF16, tag=f"in{i}", name=f"in{i}")
        bias_t = cpool.tile([128, 17], F32, tag="bias")

        def load(i):
            nc.sync.dma_start(T[i][:], big_d[:, TILE_OFF[i]:TILE_OFF[i] + TILE_LEN[i]])
        load(0)
        load(1)
        for r in range(4):
            nc.sync.dma_start(pt2t[32 * r:32 * r + 20, :],
                              pt2_d[20 * r:20 * r + 20, :])
        nc.sync.dma_start(bias_t[:], bias_d[:])
        load(2)
        load(4)
        load(3)
        load(5)
        load(6)
        load(7)

        def w0a_ap(g):
            q, r = divmod(g, 4)
            if q == 0:
                return T[0][:, 512 + 128 * r:640 + 128 * r]
            if q <= 3:
                c = 512 * (q - 1) + 128 * r
                return T[2][:, c:c + 128]
            c = 512 * (q - 4) + 128 * r
            return T[4][:, c:c + 128]

        def pt1_ap(t):
            i, c = PT1LOC[t]
            return T[i][:, c:c + PTILE]

        def pt2_ap(t, r):
            return pt2t[32 * r:32 * r + 20, ts(t, PTILE)]

        def w0bp_ap(q, r):
            return T[1][32 * r:32 * r + 20, 128 * q:128 * q + 128]

        def w1bd_ap(g):
            return T[3][:, 64 * g:64 * g + 64]

        def w2_ap(p):
            return T[5][:, 32 * p:32 * p + 32]

        def b1_ap(p):
            return bias_t[:, p:p + 1]

        b2_ap = lambda: bias_t[:, 16:17]

        def emit_l0(t):
            # ---- layer 0: 8 quads of 4 filter-groups; two (128,1024) PSUM
            # tiles per quad. chunk1 = K rows 0..127 full-array; chunk2
            # (K rows 128..146 + bias row) = 4-way row-tiled concurrent
            # matmuls (4 bands x 4 banks). Bias rides in the matmul.
            h0 = []       # 16 tiles (128,1024): groups (2j, 2j+1)
            for q in range(8):
                psA = l0pool.tile([128, 2 * PTILE], F32, tag="l0")
                psB = l0pool.tile([128, 2 * PTILE], F32, tag="l0")
                for r in range(4):
                    ps = psA if r < 2 else psB
                    dst = ps[:, ts(r % 2, PTILE)]
                    nc.tensor.matmul(dst, w0a_ap(4 * q + r), pt1_ap(t),
                                     start=True, stop=False)
                for r in range(4):
                    ps = psA if r < 2 else psB
                    dst = ps[:, ts(r % 2, PTILE)]
                    nc.tensor.matmul(dst, w0bp_ap(q, r), pt2_ap(t, r),
                                     start=False, stop=True,
                                     tile_position=(32 * r, 0))
                if t == 0 and q < 6:
                    # keep the PE (and the HAM clock gate) busy while tile-0
                    # quads wait on their input DMAs
                    for _ in range(2):
                        nc.tensor.matmul(wups[:], wu[:, 0:128], wu[:],
                                         start=True, stop=True)
                for j, ps in ((2 * q, psA), (2 * q + 1, psB)):
                    h = h0pool.tile([128, 2 * PTILE], BF16, tag="h0")
                    # both engines evacuate one half each (the split MUST sit
                    # on the PSUM bank boundary -- each engine touches one
                    # bank): the pair frees ~40% sooner, easing the rotation
                    nc.scalar.activation(h[:, 0:PTILE], ps[:, 0:PTILE], Relu)
                    nc.vector.tensor_scalar_max(h[:, PTILE:], ps[:, PTILE:], 0.0)
                    h0.append(h)
            return h0

        def emit_l12(t, h0):
            pix = ts(t, PTILE)
            # ---- layer 1: per pair of groups, block-diag W1 (128, 64), two
            # col-tiled matmuls fill the two partition halves of one PSUM bank
            h1 = []
            for p in range(16):
                ps = l12pool.tile([128, PTILE], F32, tag="l12")
                nc.tensor.matmul(ps[0:64, :], w1bd_ap(2 * p),
                                 h0[p][:, 0:PTILE], start=True, stop=True)
                nc.tensor.matmul(ps[64:128, :], w1bd_ap(2 * p + 1),
                                 h0[p][:, PTILE:], start=True, stop=True)
                hh = h1pool.tile([128, PTILE], BF16, tag="h1")
                if p % 2 == 0 or p == 15:
                    # 9/7 split: ACT is slightly cheaper per element, so it
                    # takes one extra h1 to equalize the engines
                    nc.scalar.activation(hh[:], ps[:], Relu, bias=b1_ap(p))
                else:
                    nc.vector.tensor_scalar(hh[:], ps[:], b1_ap(p), 0.0, Add, Max)
                h1.append(hh)
            # ---- layer 2: 4 blocks of 32 filters; q-major order so the 4
            # blocks' matmuls hit disjoint PE column groups concurrently
            ps2 = l12pool.tile([128, PTILE], F32, tag="l12")
            for qq in range(4):
                for jj in range(4):
                    p = 4 * jj + qq
                    nc.tensor.matmul(ps2[32 * jj:32 * jj + 32, :],
                                     w2_ap(p), h1[p][:],
                                     start=(qq == 0), stop=(qq == 3),
                                     tile_position=(0, 32 * jj))
            ot = opool.tile([128, PTILE], BF16, tag="o")
            if t == NT - 1:
                # split the final evac + store so the kernel-ending DMA is
                # small (the postamble waits on its completion semaphore)
                HALF = PTILE // 2
                nc.scalar.activation(ot[:, 0:HALF], ps2[:, 0:HALF], Relu, bias=b2_ap())
                nc.sync.dma_start(out[:, t * PTILE:t * PTILE + HALF], ot[:, 0:HALF])
                nc.scalar.activation(ot[:, HALF:], ps2[:, HALF:], Relu, bias=b2_ap())
                nc.sync.dma_start(out[:, t * PTILE + HALF:(t + 1) * PTILE], ot[:, HALF:])
            else:
                nc.scalar.activation(ot[:], ps2[:], Relu, bias=b2_ap())
                nc.sync.dma_start(out[:, pix], ot[:])

        for t in range(NT):
            emit_l12(t, emit_l0(t))


_COMPILED = None


def _get_compiled():
    global _COMPILED
    if _COMPILED is None:
        import time as _time
        t0 = _time.time()
        nc = bacc.Bacc("TRN2", target_bir_lowering=False, debug=False,
                       num_devices=NCORES)
        with tile.TileContext(nc) as tc:
            _body(tc)
        t1 = _time.time()
        nc.compile()
        t2 = _time.time()
        print(f"[kernel] tile build+schedule {t1 - t0:.1f}s, bacc compile {t2 - t1:.1f}s",
              flush=True)
        _COMPILED = nc
    return _COMPILED


# ----------------------------------------------------------------------------
# public entry point
# ----------------------------------------------------------------------------

def kernel(x, w0, b0, w1, b1, w2, b2, _trace=False):
    x = np.asarray(x, np.float32)
    shared, bias = _pack_shared(w0, b0, w1, b1, w2, b2)

    in_maps = []
    for k in range(NCORES):
        PT = _im2col_T(x[BPC * k:BPC * (k + 1)])
        big = shared.copy()
        pt1 = PT[:128].astype(NPBF16)
        for t in range(NT):
            big[:, PT1COL[t]:PT1COL[t] + PTILE] = pt1[:, ts_np(t)]
        in_maps.append({"big": big, "pt2": _pack_pt2(PT), "bias": bias})

    import time as _time
    nc = _get_compiled()
    t0 = _time.time()
    res = bass_utils.run_bass_kernel_spmd(
        nc, in_maps, core_ids=list(range(NCORES)), trace=_trace)
    print(f"[kernel] run_bass_kernel_spmd {_time.time() - t0:.1f}s", flush=True)

    outs = []
    for k in range(NCORES):
        oc = np.asarray(res.results[k]["out"], np.float32)   # (128, 4096)
        outs.append(oc.reshape(F, BPC, H, W).transpose(1, 2, 3, 0))
    full = np.concatenate(outs, axis=0).astype(np.float32)
    if _trace:
        return full, res
    return full


def ts_np(t):
    return slice(t * PTILE, (t + 1) * PTILE)
